# revision 36
# baseline (speedup 1.0000x reference)
"""Trainium2 Bass kernel for nn_CrossAttention (B=2, N=2048, C=1024, H=16).

Sharding: 16 heads / 8 cores = 2 heads per core (both batches on every
core).  Each core computes its heads' Q/K/V projections with the matching
128-row slice of Wq/Wk/Wv, full attention for its 4 (batch, head) pairs,
and a partial output projection against its 128-column slice of Wp.  The
host sums the 8 partial projections (the tensor-parallel all-reduce) and
adds the bias.

Schedule: a 128-step software pipeline (8 units of (batch, 512 queries)
x 16 key-chunks of 128 keys).  Per step: scores = 2 row-tiled concurrent
matmuls (K=64 per head -> PE tiles 0-63 / 64-127), one [128,1024] Exp on
the Scalar engine (the pacing resource, ~1.15us/step), then attnv (2
col-tiled concurrent matmuls) + softmax-denominator (2 col-tiled M=1
matmuls) lagging LAG steps behind.  Q/K projections, direct key-major V
projection (lhsT = x tile, rhs = Wv chunk -- no PE transposes), and the
output projection are woven into the remaining PE slack via a
deadline-ordered FIFO queue with a per-step PE-nanosecond budget so the
PE stays dense (HAM K=8/8) without outrunning the 2-deep scores ring.

Each x piece (8 contraction chunks x 512 positions) loads as ONE 3D-AP
DMA [128, 8, 512] so the 16 SDMA engines split it; weights load on the
Activation engine's separate HW-DGE ring so the prologue overlaps.

PSUM (8 banks of 2KB/partition):
  sc pool  2 x [128,1024] f32 = 4 banks  (scores ring: h0 cols 0:512, h1 512:1024)
  o  pool  2 x [128, 512] f32 = 2 banks  (attnv accumulators, per unit)
  d  pool  1 x [128, 512] f32 = 1 bank   (denominator rows 0 / 64)
  qp pool  1 x [128, 512] f32 = 1 bank   (qkv / vpos / proj accumulator)

On-device layouts (per core, fp16 matmul operands / fp32 PSUM):
  xq/xt   [1024 ch, 4096 pos]   channel-major inputs (host pre-transposed)
  x tile  [128, 8*512]          chunk kc at cols kc*512..+512
  q2T/k2T 4 x [128, 1024]       per (batch, pos-half); rows 0-63 head0 dims
  vpos    2 x [128, 2048]       per batch, key-major V: block kb at cols
                                kb*128..+128, within block h*64+d
  st      [128 keys, 1024]      exp(scores) per step, h0 | h1, fp16
  outT    [128 ch, 512 pos]     normalized attention output per unit
  out_p   [2, 2048, 1024] f16   partial projection (summed on host)
"""

import os
import sys

for _p in ("/opt/trn_rl_repo", os.path.expanduser("~/.axon_site/_ro/trn_rl_repo")):
    if os.path.isdir(_p) and _p not in sys.path:
        sys.path.insert(0, _p)

import numpy as np

import concourse.bacc as bacc
import concourse.mybir as mybir
import concourse.tile as tile
from concourse.bass_utils import run_bass_kernel_spmd

F16 = mybir.dt.float16
F32 = mybir.dt.float32
AF = mybir.ActivationFunctionType

B, N, C, H, D = 2, 2048, 1024, 16, 64
NCORES = 8
SCALE = float(D) ** -0.5
STEP_NS = 1300 # per-step PE budget the weave fills up to


def lag_of(a):
    # attnv-stream lag behind the scores/exp stream: long at the start
    # (u0's key-major V is still being produced) and bumped across the
    # b0->b1 switch (b1's V is produced mid-stream); e(a)=a+lag_of(a)
    # stays non-decreasing so emission remains in order
    if a < 24:
        return max(4, 16 - (2 * a) // 3)
    if a < 56:
        return 4
    if a < 72:
        return min(8, 4 + (a - 55))
    if a < 80:
        return max(4, 8 - (a - 72) // 2)
    return 4

TRACE = False
LAST_EXEC_NS = None
LAST_RESULTS = None

_COMPILED_NC = None


def _emit(nc):
    xq = nc.dram_tensor("xq", [C, B * N], F16, kind="ExternalInput")
    xt = nc.dram_tensor("xt", [C, B * N], F16, kind="ExternalInput")
    wq = nc.dram_tensor("wq", [128, 1024], F16, kind="ExternalInput")
    wk = nc.dram_tensor("wk", [128, 1024], F16, kind="ExternalInput")
    wv = nc.dram_tensor("wv", [128, 1024], F16, kind="ExternalInput")
    wp = nc.dram_tensor("wp", [128, C], F16, kind="ExternalInput")
    outp = nc.dram_tensor("out_p", [B, N, C], F16, kind="ExternalOutput")

    with tile.TileContext(nc) as tc:
        with (
            tc.tile_pool(name="consts", bufs=1) as cpool,
            tc.tile_pool(name="xsq", bufs=3) as xsq,
            tc.tile_pool(name="xst", bufs=4) as xst,
            tc.tile_pool(name="xsq0", bufs=8) as xsq0,
            tc.tile_pool(name="xst0", bufs=8) as xst0,
            tc.tile_pool(name="act", bufs=1) as apool,
            tc.tile_pool(name="stp", bufs=19) as stp,
            tc.tile_pool(name="ob", bufs=4) as obp,
            tc.tile_pool(name="rc", bufs=2) as rcpool,
            tc.tile_pool(name="pe", bufs=2) as pep,
            tc.tile_pool(name="sc", bufs=2, space="PSUM") as scp,
            tc.tile_pool(name="o", bufs=1, space="PSUM") as op,
            tc.tile_pool(name="d", bufs=1, space="PSUM") as dp,
            tc.tile_pool(name="qp", bufs=2, space="PSUM") as qpp,
        ):
            # ---- persistent tiles -----------------------------------------
            w_sb = {}
            for name in ("wq", "wk", "wv"):
                w_sb[name] = cpool.tile([128, 1024], F16, tag=name, name=f"w_{name}")
            wp_sb = cpool.tile([128, C], F16, tag="wp")
            ones_sb = cpool.tile([128, 64], F16, tag="ones")
            warm = cpool.tile([128, 16], F16, tag="warm")
            warm2 = cpool.tile([128, 16], F16, tag="warm2")

            q2T = {}
            k2T = {}
            vpos = {}
            for b in range(2):
                vpos[b] = apool.tile([128, 2048], F16, tag=f"vp{b}", name=f"vpos{b}")
                for ph in range(2):
                    q2T[(b, ph)] = apool.tile(
                        [128, 1024], F16, tag=f"q{b}{ph}", name=f"q2T{b}{ph}"
                    )
                    k2T[(b, ph)] = apool.tile(
                        [128, 1024], F16, tag=f"k{b}{ph}", name=f"k2T{b}{ph}"
                    )

            st = {}    # step -> exp(scores) tile [128, 1024] (h0 | h1)
            sc = {}    # step -> scores psum tile
            ps_o = {}  # unit -> attnv accumulator
            ps_d = {}  # unit -> denominator psum
            outT = {}  # unit -> normalized attention output [128, 512]
            x_map = {}  # (which, b, ph, qc) -> x tile (or list of chunk tiles)

            def load_x(which, b, ph, qc, eng=None):
                # one 3D DMA: [128, 8 chunks, 512 positions]
                key = (which, b, ph, qc)
                if key in x_map:
                    return
                pool = xsq if which == "q" else xst
                src = xq if which == "q" else xt
                x_t = pool.tile([128, 4096], F16, tag="x", name=f"x{b}{ph}{qc}")
                cols = slice(
                    b * 2048 + ph * 1024 + qc * 512,
                    b * 2048 + ph * 1024 + (qc + 1) * 512,
                )
                dst = x_t[:].rearrange("p (kc c) -> p kc c", c=512)
                srcap = src[0:1024, cols].rearrange("(kc p) c -> p kc c", p=128)
                (eng or nc.sync).dma_start(dst, srcap)
                x_map[key] = x_t

            def xslice(x_t, kc, lo=0, hi=512):
                if isinstance(x_t, list):
                    return x_t[kc][:, lo:hi]
                return x_t[:, kc * 512 + lo : kc * 512 + hi]

            # ---- weave piece generators (yield = estimated PE ns) ---------
            def qk_piece(which, b, ph, qc):
                load_x(which, b, ph, qc)
                x_t = x_map[(which, b, ph, qc)]
                yield 0
                ps = qpp.tile([128, 512], F32, tag="qk", name=f"ps{which}{b}{ph}{qc}")
                for kc in range(8):
                    nc.tensor.matmul(
                        ps[:],
                        lhsT=w_sb["w" + which][:, kc * 128 : (kc + 1) * 128],
                        rhs=xslice(x_t, kc),
                        start=(kc == 0),
                        stop=(kc == 7),
                    )
                    if kc % 2 == 1:
                        yield 426
                dst = (q2T if which == "q" else k2T)[(b, ph)]
                nc.vector.tensor_copy(dst[:, qc * 512 : (qc + 1) * 512], ps[:])

            def vpos_group(b, g):
                # vpos[b][:, g*512:+512]: 4 key-blocks of [128 keys, 128 dims]
                x_t = x_map[("k", b, g // 2, g % 2)]
                ps = qpp.tile([128, 512], F32, tag="qk", name=f"psv{b}{g}")
                for blk in range(4):
                    bs = slice(blk * 128, (blk + 1) * 128)
                    for kc in range(8):
                        nc.tensor.matmul(
                            ps[:, bs],
                            lhsT=xslice(x_t, kc, blk * 128, (blk + 1) * 128),
                            rhs=w_sb["wv"][:, kc * 128 : (kc + 1) * 128],
                            start=(kc == 0),
                            stop=(kc == 7),
                        )
                        if kc % 4 == 3:
                            yield 400
                nc.vector.tensor_copy(vpos[b][:, g * 512 : (g + 1) * 512], ps[:])

            def proj_unit(u):
                # out_p[b, qs*512 .. +512, :] partial projection of outT[u],
                # staged in one [128, 4096] tile and written with ONE 3D DMA
                b, qs = u // 4, u % 4
                ev = pep.tile([128, 4096], F16, tag="pe", name=f"pe{u}")
                for pm in range(4):
                    for half in range(2):
                        ps = qpp.tile([128, 512], F32, tag="qk", name=f"pp{u}{pm}{half}")
                        nc.tensor.matmul(
                            ps[:],
                            lhsT=outT[u][:, pm * 128 : (pm + 1) * 128],
                            rhs=wp_sb[:, half * 512 : (half + 1) * 512],
                            start=True,
                            stop=True,
                        )
                        nc.vector.tensor_copy(
                            ev[:, pm * 1024 + half * 512 : pm * 1024 + (half + 1) * 512],
                            ps[:],
                        )
                        yield 550
                if u == 7:
                    # last unit: split the write so the final DMA is small
                    for pm in range(4):
                        rows = slice(qs * 512 + pm * 128, qs * 512 + (pm + 1) * 128)
                        nc.sync.dma_start(
                            outp[b, rows, :],
                            ev[:, pm * 1024 : (pm + 1) * 1024],
                        )
                else:
                    rows = slice(qs * 512, (qs + 1) * 512)
                    dst = outp[b, rows, :].rearrange("(pm p) c -> p pm c", p=128)
                    nc.sync.dma_start(
                        dst, ev[:].rearrange("p (pm c) -> p pm c", c=1024)
                    )

            # ---- weave queue: FIFO by deadline, ns-budgeted pump ----------
            weave = []   # [deadline, seq, key, gen, min_start]
            done = set()
            active = [None]
            seqc = [0]
            now = [0]    # current pipeline step (for min_start gating)

            def enqueue(deadline, key, gen, min_start=0):
                seqc[0] += 1
                weave.append([deadline, seqc[0], key, gen, min_start])
                weave.sort(key=lambda it: (it[0], it[1]))

            def pump_one(force_key=None):
                # returns est ns emitted, or None if nothing eligible
                if active[0] is None:
                    pick = None
                    for idx, item in enumerate(weave):
                        if force_key is not None or item[4] <= now[0]:
                            pick = idx
                            break
                    if pick is None:
                        return None
                    item = weave.pop(pick)
                    active[0] = [item[2], item[3]]
                key, gen = active[0]
                try:
                    return next(gen)
                except StopIteration:
                    done.add(key)
                    active[0] = None
                    return 0

            debt = [0]

            def pump(budget):
                budget -= debt[0]
                debt[0] = max(0, -budget)
                while budget > 0:
                    got = pump_one()
                    if got is None:
                        return
                    budget -= max(got, 1)

            def drain_until(key):
                # forced drains charge the next steps' pump budgets
                while key not in done:
                    got = pump_one(force_key=key)
                    if got is None:
                        raise RuntimeError(f"weave exhausted before {key}")
                    debt[0] += got

            # ---- attention streams ----------------------------------------
            def scores_exp(s):
                u, m = s // 16, s % 16
                b, qs = u // 4, u % 4
                t = scp.tile([128, 1024], F32, tag="sc", name=f"sc{s}")
                kt = k2T[(b, m // 8)]
                ms = slice((m % 8) * 128, (m % 8 + 1) * 128)
                for h in range(2):
                    hp = slice(h * 64, (h + 1) * 64)
                    nc.tensor.matmul(
                        t[:, h * 512 : (h + 1) * 512],
                        lhsT=kt[hp, ms],
                        rhs=q2T[(b, qs // 2)][hp, (qs % 2) * 512 : (qs % 2 + 1) * 512],
                        start=True,
                        stop=True,
                    )
                sc[s] = t
                e = stp.tile([128, 1024], F16, tag="st", name=f"st{s}")
                nc.scalar.activation(e[:], t[:], AF.Exp, scale=SCALE)
                st[s] = e

            def attnv_denom(a):
                u, m = a // 16, a % 16
                b = u // 4
                if m == 0:
                    ps_o[u] = op.tile([128, 512], F32, tag="o", name=f"o{u}")
                    ps_d[u] = dp.tile([128, 512], F32, tag="pd", name=f"d{u}")
                po, pd = ps_o[u], ps_d[u]
                e = st.pop(a)
                kw = dict(start=(m == 0), stop=(m == 15))
                for h in range(2):
                    nc.tensor.matmul(
                        po[h * 64 : (h + 1) * 64, :],
                        lhsT=vpos[b][:, m * 128 + h * 64 : m * 128 + (h + 1) * 64],
                        rhs=e[:, h * 512 : (h + 1) * 512],
                        **kw,
                    )
                for h in range(2):
                    # M=64 all-ones lhsT: every output row gets the key-sum,
                    # so the denominator lands pre-broadcast across the head's
                    # 64 partitions (no shuffle pass needed in normalize)
                    nc.tensor.matmul(
                        pd[h * 64 : (h + 1) * 64, :],
                        lhsT=ones_sb[:, 0:64],
                        rhs=e[:, h * 512 : (h + 1) * 512],
                        **kw,
                    )

            def normalize(u):
                po, pd = ps_o.pop(u), ps_d.pop(u)
                rc = rcpool.tile([128, 512], F32, tag="rc", name=f"rc{u}")
                nc.vector.reciprocal_approx_fast(rc[:], pd[:])
                # outT ring is 4 deep: proj(u-4) must be fully emitted before
                # the mul below reuses its outT slot
                if u >= 4 and ("proj", u - 4) not in done:
                    drain_until(("proj", u - 4))
                ot = obp.tile([128, 512], F16, tag="outT", name=f"outT{u}")
                nc.vector.tensor_mul(ot[:], po[:], rc[:])
                outT[u] = ot

            # ---- prologue -------------------------------------------------
            # First k/q pieces load as 8 per-chunk DMAs each so their matmuls
            # chase the DMA stream: xt chunks on the SP HW-DGE ring, weights
            # + xq chunks on the Activation ring (transfers overlap).  PE
            # warmup matmuls (scrap output, zero weights) fill the DMA wait
            # so the HAM clock gate reaches K=8/8 before the real work.
            def load_chunks(pool, tag, src, eng):
                # 4 x 256KB chunk DMAs, each [128, 2 kc, 512]
                tiles = []
                for j in range(4):
                    t_ = pool.tile([128, 1024], F16, tag=tag, name=f"{tag}c{j}")
                    dst = t_[:].rearrange("p (kc c) -> p kc c", c=512)
                    srcap = src[j * 256 : (j + 1) * 256, 0:512].rearrange(
                        "(kc p) c -> p kc c", p=128
                    )
                    eng.dma_start(dst, srcap)
                    tiles.append(t_)
                return tiles

            nc.sync.dma_start(w_sb["wk"][:], wk[:])
            nc.sync.dma_start(w_sb["wq"][:], wq[:])
            xq0 = load_chunks(xsq0, "xq0", xq, nc.scalar)
            xt0 = load_chunks(xst0, "xt0", xt, nc.sync)
            nc.scalar.dma_start(w_sb["wv"][:], wv[:])
            nc.scalar.dma_start(wp_sb[:], wp[:])
            x_map[("k", 0, 0, 0)] = [
                xt0[kc // 2][:, (kc % 2) * 512 : (kc % 2 + 1) * 512] for kc in range(8)
            ]
            nc.vector.memset(ones_sb[:], 1.0)
            nc.vector.memset(warm[:], 0.0)
            nc.scalar.activation(warm2[:], warm[:], AF.Exp, scale=SCALE)

            warm_ps = op.tile([128, 512], F32, tag="o", name="warmps")

            def warm_mm():
                nc.tensor.matmul(
                    warm_ps[0:16, :],
                    lhsT=warm[:, 0:16],
                    rhs=vpos[1][:, 0:512],
                    start=True,
                    stop=True,
                )

            def first_pieces():
                # k and q chunk matmuls interleaved in expected DMA-arrival
                # order (xt chunks on the sync ring land slightly earlier)
                for _ in range(6):
                    warm_mm()
                ps = qpp.tile([128, 512], F32, tag="qk", name="psk000")
                ps2 = qpp.tile([128, 512], F32, tag="qk", name="psq000")
                for j in range(4):
                    for kc in (2 * j, 2 * j + 1):
                        nc.tensor.matmul(
                            ps[:],
                            lhsT=w_sb["wk"][:, kc * 128 : (kc + 1) * 128],
                            rhs=xt0[j][:, (kc % 2) * 512 : (kc % 2 + 1) * 512],
                            start=(kc == 0),
                            stop=(kc == 7),
                        )
                    for kc in (2 * j, 2 * j + 1):
                        nc.tensor.matmul(
                            ps2[:],
                            lhsT=w_sb["wq"][:, kc * 128 : (kc + 1) * 128],
                            rhs=xq0[j][:, (kc % 2) * 512 : (kc % 2 + 1) * 512],
                            start=(kc == 0),
                            stop=(kc == 7),
                        )
                nc.vector.tensor_copy(k2T[(0, 0)][:, 0:512], ps[:])
                nc.vector.tensor_copy(q2T[(0, 0)][:, 0:512], ps2[:])

            first_pieces()
            done.add(("k", 0, 0))
            done.add(("q", 0, 0))

            # fence the SP DMA ring on the first k-piece eviction so the
            # eager x loads below don't steal HBM bandwidth from the
            # prologue chunk transfers (ring issues are FIFO)
            nc.sync.dma_start(warm[0:1, 0:1], k2T[(0, 0)][0:1, 0:1])

            # remaining pieces, deadline-ordered (step index of first use),
            # x loads issued ~10-16 steps ahead, piece emission gated ~4
            # steps after its DMA issue so queued matmuls don't camp on
            # the PE queue ahead of scores
            x_sched = []
            for b in range(2):
                base = b * 64
                ahead = 10 if b == 0 else 16
                for kb in range(4):
                    vdl = base + kb * 4 + lag_of(base + kb * 4)
                    if (b, kb) != (0, 0):
                        ds = max(1, base + kb * 4 - ahead)
                        enqueue(base + kb * 4, ("k", b, kb),
                                qk_piece("k", b, kb // 2, kb % 2), ds + 4)
                        x_sched.append((ds, "k", b, kb // 2, kb % 2))
                        vms = ds + 6
                    else:
                        vms = 0
                    enqueue(vdl, ("v", b, kb), vpos_group(b, kb), vms)
                for qs in range(4):
                    if (b, qs) != (0, 0):
                        ds = max(1, base + qs * 16 - ahead)
                        if (b, qs) == (0, 1):
                            ds = 2
                        enqueue(base + qs * 16, ("q", b, qs),
                                qk_piece("q", b, qs // 2, qs % 2), ds + 4)
                        x_sched.append((ds, "q", b, qs // 2, qs % 2))
            x_sched.sort(key=lambda it: it[0])

            # ---- main pipeline --------------------------------------------
            def attn_step(a):
                u, m = a // 16, a % 16
                if m % 4 == 0:
                    drain_until(("v", u // 4, m // 4))
                attnv_denom(a)
                if m == 15:
                    normalize(u)
                    enqueue(min((u + 2) * 16, 124), ("proj", u), proj_unit(u))

            attn_ptr = [0]

            for s in range(128):
                u, m = s // 16, s % 16
                b, qs = u // 4, u % 4
                now[0] = s
                while x_sched and x_sched[0][0] <= s:
                    _, which, lb, lph, lqc = x_sched.pop(0)
                    load_x(which, lb, lph, lqc)
                if m == 0 and (b, qs) != (0, 0):
                    drain_until(("q", b, qs))
                if m % 4 == 0 and (b, m // 4) != (0, 0):
                    drain_until(("k", b, m // 4))
                scores_exp(s)
                n_attn = 0
                while attn_ptr[0] < 128 and attn_ptr[0] + lag_of(attn_ptr[0]) <= s:
                    attn_step(attn_ptr[0])
                    attn_ptr[0] += 1
                    n_attn += 1
                pump(STEP_NS - 213 - 426 * n_attn)

            # ---- epilogue -------------------------------------------------
            now[0] = 10 ** 9
            while attn_ptr[0] < 128:
                attn_step(attn_ptr[0])
                attn_ptr[0] += 1
            pump(10 ** 9)
    return nc


def _get_compiled():
    global _COMPILED_NC
    if _COMPILED_NC is None:
        nc = bacc.Bacc(
            "TRN2", target_bir_lowering=False, debug=False, num_devices=NCORES
        )
        _emit(nc)
        nc.compile()
        _COMPILED_NC = nc
    return _COMPILED_NC


def _install_trace_shim():
    """Register antenv.axon_hooks NTFF hook (missing on this image)."""
    import contextlib
    import ctypes
    import types

    if "antenv.axon_hooks" in sys.modules:
        return
    try:
        import antenv
    except ImportError:
        return
    so_path = "/opt/axon/libaxon_pjrt.so"
    if not os.path.exists(so_path):
        return

    mod = types.ModuleType("antenv.axon_hooks")
    mod._hook = None
    mod.set_axon_ntff_profile_hook = lambda h: setattr(mod, "_hook", h)
    mod.get_axon_ntff_profile_hook = lambda: mod._hook

    lib = ctypes.CDLL(so_path)
    if not hasattr(lib, "axon_start_nrt_profile"):
        return
    lib.axon_start_nrt_profile.argtypes = [
        ctypes.POINTER(ctypes.c_int64),
        ctypes.c_size_t,
    ]
    lib.axon_start_nrt_profile.restype = ctypes.c_int64
    lib.axon_stop_nrt_profile.argtypes = [ctypes.c_char_p]
    lib.axon_stop_nrt_profile.restype = ctypes.c_int64

    @contextlib.contextmanager
    def _hook(output_dir, device_ids):
        import jax

        jax.devices()
        if device_ids:
            ids = (ctypes.c_int64 * len(device_ids))(*device_ids)
            rc = lib.axon_start_nrt_profile(ids, len(device_ids))
        else:
            rc = lib.axon_start_nrt_profile(None, 0)
        if rc != 0:
            raise RuntimeError(f"axon_start_nrt_profile rc={rc}")
        try:
            yield
        finally:
            n = lib.axon_stop_nrt_profile(str(output_dir).encode())
            if n < 0:
                raise RuntimeError(f"axon_stop_nrt_profile rc={n}")

    mod.set_axon_ntff_profile_hook(_hook)
    sys.modules["antenv.axon_hooks"] = mod
    antenv.axon_hooks = mod


def kernel(query, target, Wq, Wk, Wv, Wp, bp):
    global LAST_EXEC_NS, LAST_RESULTS
    query = np.asarray(query, dtype=np.float32)
    target = np.asarray(target, dtype=np.float32)
    Wq = np.asarray(Wq, dtype=np.float32)
    Wk = np.asarray(Wk, dtype=np.float32)
    Wv = np.asarray(Wv, dtype=np.float32)
    Wp = np.asarray(Wp, dtype=np.float32)
    bp = np.asarray(bp, dtype=np.float32)

    xq = np.ascontiguousarray(query.reshape(B * N, C).T).astype(np.float16)
    xt = np.ascontiguousarray(target.reshape(B * N, C).T).astype(np.float16)

    def wlayout(Wm, rows):
        # SBUF weight tile [p, kc*128 + m] = W[row0 + m, kc*128 + p]
        ws = Wm[rows, :].astype(np.float16)  # (128, 1024)
        return np.ascontiguousarray(
            ws.reshape(128, 8, 128).transpose(2, 1, 0).reshape(128, 1024)
        )

    in_maps = []
    for c in range(NCORES):
        rows = slice(c * 128, (c + 1) * 128)
        in_maps.append(
            {
                "xq": xq,
                "xt": xt,
                "wq": wlayout(Wq, rows),
                "wk": wlayout(Wk, rows),
                "wv": wlayout(Wv, rows),
                "wp": np.ascontiguousarray(Wp[:, rows].T).astype(np.float16),
            }
        )

    if TRACE:
        _install_trace_shim()

    nc = _get_compiled()
    res = run_bass_kernel_spmd(
        nc, in_maps, core_ids=list(range(NCORES)), trace=TRACE
    )
    LAST_RESULTS = res
    LAST_EXEC_NS = res.exec_time_ns

    acc = res.results[0]["out_p"].astype(np.float64)
    for c in range(1, NCORES):
        acc += res.results[c]["out_p"]
    out = acc.astype(np.float32) + bp[None, None, :]
    return out


# revision 37
# speedup vs baseline: 1.0120x; 1.0120x over previous
"""Trainium2 Bass kernel for nn_CrossAttention (B=2, N=2048, C=1024, H=16).

Sharding: 16 heads / 8 cores = 2 heads per core (both batches on every
core).  Each core computes its heads' Q/K/V projections with the matching
128-row slice of Wq/Wk/Wv, full attention for its 4 (batch, head) pairs,
and a partial output projection against its 128-column slice of Wp.  The
host sums the 8 partial projections (the tensor-parallel all-reduce) and
adds the bias.

Schedule: a 128-step software pipeline (8 units of (batch, 512 queries)
x 16 key-chunks of 128 keys).  Per step: scores = 2 row-tiled concurrent
matmuls (K=64 per head -> PE tiles 0-63 / 64-127), one [128,1024] Exp on
the Scalar engine (the pacing resource, ~1.15us/step), then attnv (2
col-tiled concurrent matmuls) + softmax-denominator (2 col-tiled M=1
matmuls) lagging LAG steps behind.  Q/K projections, direct key-major V
projection (lhsT = x tile, rhs = Wv chunk -- no PE transposes), and the
output projection are woven into the remaining PE slack via a
deadline-ordered FIFO queue with a per-step PE-nanosecond budget so the
PE stays dense (HAM K=8/8) without outrunning the 2-deep scores ring.

Each x piece (8 contraction chunks x 512 positions) loads as ONE 3D-AP
DMA [128, 8, 512] so the 16 SDMA engines split it; weights load on the
Activation engine's separate HW-DGE ring so the prologue overlaps.

PSUM (8 banks of 2KB/partition):
  sc pool  2 x [128,1024] f32 = 4 banks  (scores ring: h0 cols 0:512, h1 512:1024)
  o  pool  2 x [128, 512] f32 = 2 banks  (attnv accumulators, per unit)
  d  pool  1 x [128, 512] f32 = 1 bank   (denominator rows 0 / 64)
  qp pool  1 x [128, 512] f32 = 1 bank   (qkv / vpos / proj accumulator)

On-device layouts (per core, fp16 matmul operands / fp32 PSUM):
  xq/xt   [1024 ch, 4096 pos]   channel-major inputs (host pre-transposed)
  x tile  [128, 8*512]          chunk kc at cols kc*512..+512
  q2T/k2T 4 x [128, 1024]       per (batch, pos-half); rows 0-63 head0 dims
  vpos    2 x [128, 2048]       per batch, key-major V: block kb at cols
                                kb*128..+128, within block h*64+d
  st      [128 keys, 1024]      exp(scores) per step, h0 | h1, fp16
  outT    [128 ch, 512 pos]     normalized attention output per unit
  out_p   [2, 2048, 1024] f16   partial projection (summed on host)
"""

import os
import sys

for _p in ("/opt/trn_rl_repo", os.path.expanduser("~/.axon_site/_ro/trn_rl_repo")):
    if os.path.isdir(_p) and _p not in sys.path:
        sys.path.insert(0, _p)

import numpy as np

import concourse.bacc as bacc
import concourse.mybir as mybir
import concourse.tile as tile
from concourse.bass_utils import run_bass_kernel_spmd

F16 = mybir.dt.float16
F32 = mybir.dt.float32
AF = mybir.ActivationFunctionType

B, N, C, H, D = 2, 2048, 1024, 16, 64
NCORES = 8
SCALE = float(D) ** -0.5
STEP_NS = 1300 # per-step PE budget the weave fills up to


def lag_of(a):
    # attnv-stream lag behind the scores/exp stream: long at the start
    # (u0's key-major V is still being produced) and bumped across the
    # b0->b1 switch (b1's V is produced mid-stream); e(a)=a+lag_of(a)
    # stays non-decreasing so emission remains in order
    if a < 24:
        return max(4, 16 - (2 * a) // 3)
    if a < 56:
        return 4
    if a < 72:
        return min(8, 4 + (a - 55))
    if a < 80:
        return max(4, 8 - (a - 72) // 2)
    return 4

TRACE = False
LAST_EXEC_NS = None
LAST_RESULTS = None

_COMPILED_NC = None


def _emit(nc):
    xq = nc.dram_tensor("xq", [C, B * N], F16, kind="ExternalInput")
    xt = nc.dram_tensor("xt", [C, B * N], F16, kind="ExternalInput")
    wq = nc.dram_tensor("wq", [128, 1024], F16, kind="ExternalInput")
    wk = nc.dram_tensor("wk", [128, 1024], F16, kind="ExternalInput")
    wv = nc.dram_tensor("wv", [128, 1024], F16, kind="ExternalInput")
    wp = nc.dram_tensor("wp", [128, C], F16, kind="ExternalInput")
    outp = nc.dram_tensor("out_p", [B, N, C], F16, kind="ExternalOutput")

    with tile.TileContext(nc) as tc:
        with (
            tc.tile_pool(name="consts", bufs=1) as cpool,
            tc.tile_pool(name="xsq", bufs=3) as xsq,
            tc.tile_pool(name="xst", bufs=4) as xst,
            tc.tile_pool(name="xsq0", bufs=8) as xsq0,
            tc.tile_pool(name="xst0", bufs=8) as xst0,
            tc.tile_pool(name="act", bufs=1) as apool,
            tc.tile_pool(name="stp", bufs=19) as stp,
            tc.tile_pool(name="ob", bufs=4) as obp,
            tc.tile_pool(name="rc", bufs=2) as rcpool,
            tc.tile_pool(name="pe", bufs=2) as pep,
            tc.tile_pool(name="sc", bufs=2, space="PSUM") as scp,
            tc.tile_pool(name="o", bufs=1, space="PSUM") as op,
            tc.tile_pool(name="d", bufs=1, space="PSUM") as dp,
            tc.tile_pool(name="qp", bufs=2, space="PSUM") as qpp,
        ):
            # ---- persistent tiles -----------------------------------------
            w_sb = {}
            for name in ("wq", "wk", "wv"):
                w_sb[name] = cpool.tile([128, 1024], F16, tag=name, name=f"w_{name}")
            wp_sb = cpool.tile([128, C], F16, tag="wp")
            ones_sb = cpool.tile([128, 64], F16, tag="ones")
            warm = cpool.tile([128, 16], F16, tag="warm")
            warm2 = cpool.tile([128, 16], F16, tag="warm2")

            q2T = {}
            k2T = {}
            vpos = {}
            for b in range(2):
                vpos[b] = apool.tile([128, 2048], F16, tag=f"vp{b}", name=f"vpos{b}")
                for ph in range(2):
                    q2T[(b, ph)] = apool.tile(
                        [128, 1024], F16, tag=f"q{b}{ph}", name=f"q2T{b}{ph}"
                    )
                    k2T[(b, ph)] = apool.tile(
                        [128, 1024], F16, tag=f"k{b}{ph}", name=f"k2T{b}{ph}"
                    )

            st = {}    # step -> exp(scores) tile [128, 1024] (h0 | h1)
            sc = {}    # step -> scores psum tile
            ps_o = {}  # unit -> attnv accumulator
            ps_d = {}  # unit -> denominator psum
            outT = {}  # unit -> normalized attention output [128, 512]
            x_map = {}  # (which, b, ph, qc) -> x tile (or list of chunk tiles)

            def load_x(which, b, ph, qc, eng=None):
                # one 3D DMA: [128, 8 chunks, 512 positions]
                key = (which, b, ph, qc)
                if key in x_map:
                    return
                pool = xsq if which == "q" else xst
                src = xq if which == "q" else xt
                x_t = pool.tile([128, 4096], F16, tag="x", name=f"x{b}{ph}{qc}")
                cols = slice(
                    b * 2048 + ph * 1024 + qc * 512,
                    b * 2048 + ph * 1024 + (qc + 1) * 512,
                )
                dst = x_t[:].rearrange("p (kc c) -> p kc c", c=512)
                srcap = src[0:1024, cols].rearrange("(kc p) c -> p kc c", p=128)
                (eng or nc.sync).dma_start(dst, srcap)
                x_map[key] = x_t

            def xslice(x_t, kc, lo=0, hi=512):
                if isinstance(x_t, list):
                    return x_t[kc][:, lo:hi]
                return x_t[:, kc * 512 + lo : kc * 512 + hi]

            # ---- weave piece generators (yield = estimated PE ns) ---------
            def qk_piece(which, b, ph, qc):
                load_x(which, b, ph, qc)
                x_t = x_map[(which, b, ph, qc)]
                yield 0
                ps = qpp.tile([128, 512], F32, tag="qk", name=f"ps{which}{b}{ph}{qc}")
                for kc in range(8):
                    nc.tensor.matmul(
                        ps[:],
                        lhsT=w_sb["w" + which][:, kc * 128 : (kc + 1) * 128],
                        rhs=xslice(x_t, kc),
                        start=(kc == 0),
                        stop=(kc == 7),
                    )
                    if kc % 2 == 1:
                        yield 426
                dst = (q2T if which == "q" else k2T)[(b, ph)]
                nc.vector.tensor_copy(dst[:, qc * 512 : (qc + 1) * 512], ps[:])

            def vpos_group(b, g):
                # vpos[b][:, g*512:+512]: 4 key-blocks of [128 keys, 128 dims]
                x_t = x_map[("k", b, g // 2, g % 2)]
                ps = qpp.tile([128, 512], F32, tag="qk", name=f"psv{b}{g}")
                for blk in range(4):
                    bs = slice(blk * 128, (blk + 1) * 128)
                    for kc in range(8):
                        nc.tensor.matmul(
                            ps[:, bs],
                            lhsT=xslice(x_t, kc, blk * 128, (blk + 1) * 128),
                            rhs=w_sb["wv"][:, kc * 128 : (kc + 1) * 128],
                            start=(kc == 0),
                            stop=(kc == 7),
                        )
                        if kc % 4 == 3:
                            yield 400
                nc.vector.tensor_copy(vpos[b][:, g * 512 : (g + 1) * 512], ps[:])

            def proj_unit(u):
                # out_p[b, qs*512 .. +512, :] partial projection of outT[u],
                # staged in one [128, 4096] tile and written with ONE 3D DMA
                b, qs = u // 4, u % 4
                ev = pep.tile([128, 4096], F16, tag="pe", name=f"pe{u}")
                for pm in range(4):
                    for half in range(2):
                        ps = qpp.tile([128, 512], F32, tag="qk", name=f"pp{u}{pm}{half}")
                        nc.tensor.matmul(
                            ps[:],
                            lhsT=outT[u][:, pm * 128 : (pm + 1) * 128],
                            rhs=wp_sb[:, half * 512 : (half + 1) * 512],
                            start=True,
                            stop=True,
                        )
                        nc.vector.tensor_copy(
                            ev[:, pm * 1024 + half * 512 : pm * 1024 + (half + 1) * 512],
                            ps[:],
                        )
                        yield 550
                if u == 7:
                    # last unit: split the write so the final DMA is small
                    for pm in range(4):
                        rows = slice(qs * 512 + pm * 128, qs * 512 + (pm + 1) * 128)
                        nc.sync.dma_start(
                            outp[b, rows, :],
                            ev[:, pm * 1024 : (pm + 1) * 1024],
                        )
                else:
                    rows = slice(qs * 512, (qs + 1) * 512)
                    dst = outp[b, rows, :].rearrange("(pm p) c -> p pm c", p=128)
                    nc.sync.dma_start(
                        dst, ev[:].rearrange("p (pm c) -> p pm c", c=1024)
                    )

            # ---- weave queue: FIFO by deadline, ns-budgeted pump ----------
            weave = []   # [deadline, seq, key, gen, min_start]
            done = set()
            active = [None]
            seqc = [0]
            now = [0]    # current pipeline step (for min_start gating)

            def enqueue(deadline, key, gen, min_start=0):
                seqc[0] += 1
                weave.append([deadline, seqc[0], key, gen, min_start])
                weave.sort(key=lambda it: (it[0], it[1]))

            def pump_one(force_key=None):
                # returns est ns emitted, or None if nothing eligible
                if active[0] is None:
                    pick = None
                    for idx, item in enumerate(weave):
                        if force_key is not None or item[4] <= now[0]:
                            pick = idx
                            break
                    if pick is None:
                        return None
                    item = weave.pop(pick)
                    active[0] = [item[2], item[3]]
                key, gen = active[0]
                try:
                    return next(gen)
                except StopIteration:
                    done.add(key)
                    active[0] = None
                    return 0

            def pump(budget):
                while budget > 0:
                    got = pump_one()
                    if got is None:
                        return
                    budget -= max(got, 1)

            def drain_until(key):
                while key not in done:
                    if pump_one(force_key=key) is None:
                        raise RuntimeError(f"weave exhausted before {key}")

            # ---- attention streams ----------------------------------------
            def scores_exp(s):
                u, m = s // 16, s % 16
                b, qs = u // 4, u % 4
                t = scp.tile([128, 1024], F32, tag="sc", name=f"sc{s}")
                kt = k2T[(b, m // 8)]
                ms = slice((m % 8) * 128, (m % 8 + 1) * 128)
                for h in range(2):
                    hp = slice(h * 64, (h + 1) * 64)
                    nc.tensor.matmul(
                        t[:, h * 512 : (h + 1) * 512],
                        lhsT=kt[hp, ms],
                        rhs=q2T[(b, qs // 2)][hp, (qs % 2) * 512 : (qs % 2 + 1) * 512],
                        start=True,
                        stop=True,
                    )
                sc[s] = t
                e = stp.tile([128, 1024], F16, tag="st", name=f"st{s}")
                nc.scalar.activation(e[:], t[:], AF.Exp, scale=SCALE)
                st[s] = e

            def attnv_denom(a):
                u, m = a // 16, a % 16
                b = u // 4
                if m == 0:
                    ps_o[u] = op.tile([128, 512], F32, tag="o", name=f"o{u}")
                    ps_d[u] = dp.tile([128, 512], F32, tag="pd", name=f"d{u}")
                po, pd = ps_o[u], ps_d[u]
                e = st.pop(a)
                kw = dict(start=(m == 0), stop=(m == 15))
                for h in range(2):
                    nc.tensor.matmul(
                        po[h * 64 : (h + 1) * 64, :],
                        lhsT=vpos[b][:, m * 128 + h * 64 : m * 128 + (h + 1) * 64],
                        rhs=e[:, h * 512 : (h + 1) * 512],
                        **kw,
                    )
                for h in range(2):
                    # M=64 all-ones lhsT: every output row gets the key-sum,
                    # so the denominator lands pre-broadcast across the head's
                    # 64 partitions (no shuffle pass needed in normalize)
                    nc.tensor.matmul(
                        pd[h * 64 : (h + 1) * 64, :],
                        lhsT=ones_sb[:, 0:64],
                        rhs=e[:, h * 512 : (h + 1) * 512],
                        **kw,
                    )

            def normalize(u):
                po, pd = ps_o.pop(u), ps_d.pop(u)
                rc = rcpool.tile([128, 512], F32, tag="rc", name=f"rc{u}")
                nc.vector.reciprocal_approx_fast(rc[:], pd[:])
                # outT ring is 4 deep: proj(u-4) must be fully emitted before
                # the mul below reuses its outT slot
                if u >= 4 and ("proj", u - 4) not in done:
                    drain_until(("proj", u - 4))
                ot = obp.tile([128, 512], F16, tag="outT", name=f"outT{u}")
                nc.vector.tensor_mul(ot[:], po[:], rc[:])
                outT[u] = ot

            # ---- prologue -------------------------------------------------
            # First k/q pieces load as 8 per-chunk DMAs each so their matmuls
            # chase the DMA stream: xt chunks on the SP HW-DGE ring, weights
            # + xq chunks on the Activation ring (transfers overlap).  PE
            # warmup matmuls (scrap output, zero weights) fill the DMA wait
            # so the HAM clock gate reaches K=8/8 before the real work.
            def load_chunks(pool, tag, src, eng):
                # 4 x 256KB chunk DMAs, each [128, 2 kc, 512]
                tiles = []
                for j in range(4):
                    t_ = pool.tile([128, 1024], F16, tag=tag, name=f"{tag}c{j}")
                    dst = t_[:].rearrange("p (kc c) -> p kc c", c=512)
                    srcap = src[j * 256 : (j + 1) * 256, 0:512].rearrange(
                        "(kc p) c -> p kc c", p=128
                    )
                    eng.dma_start(dst, srcap)
                    tiles.append(t_)
                return tiles

            nc.sync.dma_start(w_sb["wk"][:], wk[:])
            nc.sync.dma_start(w_sb["wq"][:], wq[:])
            xq0 = load_chunks(xsq0, "xq0", xq, nc.scalar)
            xt0 = load_chunks(xst0, "xt0", xt, nc.sync)
            nc.scalar.dma_start(w_sb["wv"][:], wv[:])
            nc.scalar.dma_start(wp_sb[:], wp[:])
            x_map[("k", 0, 0, 0)] = [
                xt0[kc // 2][:, (kc % 2) * 512 : (kc % 2 + 1) * 512] for kc in range(8)
            ]
            nc.vector.memset(ones_sb[:], 1.0)
            nc.vector.memset(warm[:], 0.0)
            nc.scalar.activation(warm2[:], warm[:], AF.Exp, scale=SCALE)

            warm_ps = op.tile([128, 512], F32, tag="o", name="warmps")

            def warm_mm():
                nc.tensor.matmul(
                    warm_ps[0:16, :],
                    lhsT=warm[:, 0:16],
                    rhs=vpos[1][:, 0:512],
                    start=True,
                    stop=True,
                )

            def first_pieces():
                # k and q chunk matmuls interleaved in expected DMA-arrival
                # order (xt chunks on the sync ring land slightly earlier)
                for _ in range(6):
                    warm_mm()
                ps = qpp.tile([128, 512], F32, tag="qk", name="psk000")
                ps2 = qpp.tile([128, 512], F32, tag="qk", name="psq000")
                for j in range(4):
                    for kc in (2 * j, 2 * j + 1):
                        nc.tensor.matmul(
                            ps[:],
                            lhsT=w_sb["wk"][:, kc * 128 : (kc + 1) * 128],
                            rhs=xt0[j][:, (kc % 2) * 512 : (kc % 2 + 1) * 512],
                            start=(kc == 0),
                            stop=(kc == 7),
                        )
                    for kc in (2 * j, 2 * j + 1):
                        nc.tensor.matmul(
                            ps2[:],
                            lhsT=w_sb["wq"][:, kc * 128 : (kc + 1) * 128],
                            rhs=xq0[j][:, (kc % 2) * 512 : (kc % 2 + 1) * 512],
                            start=(kc == 0),
                            stop=(kc == 7),
                        )
                nc.vector.tensor_copy(k2T[(0, 0)][:, 0:512], ps[:])
                nc.vector.tensor_copy(q2T[(0, 0)][:, 0:512], ps2[:])

            first_pieces()
            done.add(("k", 0, 0))
            done.add(("q", 0, 0))

            # fence the SP DMA ring on the first k-piece eviction so the
            # eager x loads below don't steal HBM bandwidth from the
            # prologue chunk transfers (ring issues are FIFO)
            nc.sync.dma_start(warm[0:1, 0:1], k2T[(0, 0)][0:1, 0:1])

            # remaining pieces, deadline-ordered (step index of first use),
            # x loads issued ~10-16 steps ahead, piece emission gated ~4
            # steps after its DMA issue so queued matmuls don't camp on
            # the PE queue ahead of scores
            x_sched = []
            for b in range(2):
                base = b * 64
                ahead = 10 if b == 0 else 16
                for kb in range(4):
                    vdl = base + kb * 4 + lag_of(base + kb * 4)
                    if (b, kb) != (0, 0):
                        ds = max(1, base + kb * 4 - ahead)
                        enqueue(base + kb * 4, ("k", b, kb),
                                qk_piece("k", b, kb // 2, kb % 2), ds + 4)
                        x_sched.append((ds, "k", b, kb // 2, kb % 2))
                        vms = ds + 6
                    else:
                        vms = 0
                    enqueue(vdl, ("v", b, kb), vpos_group(b, kb), vms)
                for qs in range(4):
                    if (b, qs) != (0, 0):
                        ds = max(1, base + qs * 16 - ahead)
                        if (b, qs) == (0, 1):
                            ds = 2
                        enqueue(base + qs * 16, ("q", b, qs),
                                qk_piece("q", b, qs // 2, qs % 2), ds + 4)
                        x_sched.append((ds, "q", b, qs // 2, qs % 2))
            x_sched.sort(key=lambda it: it[0])

            # ---- main pipeline --------------------------------------------
            def attn_step(a):
                u, m = a // 16, a % 16
                if m % 4 == 0:
                    drain_until(("v", u // 4, m // 4))
                attnv_denom(a)
                if m == 15:
                    normalize(u)
                    enqueue(min((u + 2) * 16, 124), ("proj", u), proj_unit(u))

            attn_ptr = [0]

            for s in range(128):
                u, m = s // 16, s % 16
                b, qs = u // 4, u % 4
                now[0] = s
                while x_sched and x_sched[0][0] <= s:
                    _, which, lb, lph, lqc = x_sched.pop(0)
                    load_x(which, lb, lph, lqc)
                if m == 0 and (b, qs) != (0, 0):
                    drain_until(("q", b, qs))
                if m % 4 == 0 and (b, m // 4) != (0, 0):
                    drain_until(("k", b, m // 4))
                scores_exp(s)
                n_attn = 0
                while attn_ptr[0] < 128 and attn_ptr[0] + lag_of(attn_ptr[0]) <= s:
                    attn_step(attn_ptr[0])
                    attn_ptr[0] += 1
                    n_attn += 1
                pump(STEP_NS - 213 - 426 * n_attn)

            # ---- epilogue -------------------------------------------------
            now[0] = 10 ** 9
            while attn_ptr[0] < 128:
                attn_step(attn_ptr[0])
                attn_ptr[0] += 1
            pump(10 ** 9)
    return nc


def _get_compiled():
    global _COMPILED_NC
    if _COMPILED_NC is None:
        nc = bacc.Bacc(
            "TRN2", target_bir_lowering=False, debug=False, num_devices=NCORES
        )
        _emit(nc)
        nc.compile()
        _COMPILED_NC = nc
    return _COMPILED_NC


def _install_trace_shim():
    """Register antenv.axon_hooks NTFF hook (missing on this image)."""
    import contextlib
    import ctypes
    import types

    if "antenv.axon_hooks" in sys.modules:
        return
    try:
        import antenv
    except ImportError:
        return
    so_path = "/opt/axon/libaxon_pjrt.so"
    if not os.path.exists(so_path):
        return

    mod = types.ModuleType("antenv.axon_hooks")
    mod._hook = None
    mod.set_axon_ntff_profile_hook = lambda h: setattr(mod, "_hook", h)
    mod.get_axon_ntff_profile_hook = lambda: mod._hook

    lib = ctypes.CDLL(so_path)
    if not hasattr(lib, "axon_start_nrt_profile"):
        return
    lib.axon_start_nrt_profile.argtypes = [
        ctypes.POINTER(ctypes.c_int64),
        ctypes.c_size_t,
    ]
    lib.axon_start_nrt_profile.restype = ctypes.c_int64
    lib.axon_stop_nrt_profile.argtypes = [ctypes.c_char_p]
    lib.axon_stop_nrt_profile.restype = ctypes.c_int64

    @contextlib.contextmanager
    def _hook(output_dir, device_ids):
        import jax

        jax.devices()
        if device_ids:
            ids = (ctypes.c_int64 * len(device_ids))(*device_ids)
            rc = lib.axon_start_nrt_profile(ids, len(device_ids))
        else:
            rc = lib.axon_start_nrt_profile(None, 0)
        if rc != 0:
            raise RuntimeError(f"axon_start_nrt_profile rc={rc}")
        try:
            yield
        finally:
            n = lib.axon_stop_nrt_profile(str(output_dir).encode())
            if n < 0:
                raise RuntimeError(f"axon_stop_nrt_profile rc={n}")

    mod.set_axon_ntff_profile_hook(_hook)
    sys.modules["antenv.axon_hooks"] = mod
    antenv.axon_hooks = mod


def kernel(query, target, Wq, Wk, Wv, Wp, bp):
    global LAST_EXEC_NS, LAST_RESULTS
    query = np.asarray(query, dtype=np.float32)
    target = np.asarray(target, dtype=np.float32)
    Wq = np.asarray(Wq, dtype=np.float32)
    Wk = np.asarray(Wk, dtype=np.float32)
    Wv = np.asarray(Wv, dtype=np.float32)
    Wp = np.asarray(Wp, dtype=np.float32)
    bp = np.asarray(bp, dtype=np.float32)

    xq = np.ascontiguousarray(query.reshape(B * N, C).T).astype(np.float16)
    xt = np.ascontiguousarray(target.reshape(B * N, C).T).astype(np.float16)

    def wlayout(Wm, rows):
        # SBUF weight tile [p, kc*128 + m] = W[row0 + m, kc*128 + p]
        ws = Wm[rows, :].astype(np.float16)  # (128, 1024)
        return np.ascontiguousarray(
            ws.reshape(128, 8, 128).transpose(2, 1, 0).reshape(128, 1024)
        )

    in_maps = []
    for c in range(NCORES):
        rows = slice(c * 128, (c + 1) * 128)
        in_maps.append(
            {
                "xq": xq,
                "xt": xt,
                "wq": wlayout(Wq, rows),
                "wk": wlayout(Wk, rows),
                "wv": wlayout(Wv, rows),
                "wp": np.ascontiguousarray(Wp[:, rows].T).astype(np.float16),
            }
        )

    if TRACE:
        _install_trace_shim()

    nc = _get_compiled()
    res = run_bass_kernel_spmd(
        nc, in_maps, core_ids=list(range(NCORES)), trace=TRACE
    )
    LAST_RESULTS = res
    LAST_EXEC_NS = res.exec_time_ns

    acc = res.results[0]["out_p"].astype(np.float64)
    for c in range(1, NCORES):
        acc += res.results[c]["out_p"]
    out = acc.astype(np.float32) + bp[None, None, :]
    return out
